# revision 1
# baseline (speedup 1.0000x reference)
"""Trainium2 Bass kernel for per-node temporal graph conv (LCN).

Math (matches the reference): for each node v with neighbor list idx[v]
(chain graph: v-1, v, v+1, masked at the ends),
    out[n,o,v,t] = b[v,o] + sum_{k,c,kt} x_pad[n,c,idx[v,k],t+kt] * Wm[v,o,c,k,kt]

Strategy: data-parallel over batch N across 8 cores (2 samples each);
weights/bias replicated. Per sample, x is laid out in SBUF as 13 "node
pair" blocks of 514 columns (512 + 2 temporal zero pads):
    partitions  0- 63: node 2j-1 (odd nodes; block 0 holds a zero ghost)
    partitions 64-127: node 2j   (even nodes)
so one [128, 512] slice at column offset j*514+kt stacks two adjacent
nodes' time-shifted frames on the contraction dim. Outputs are computed
per node pair (v=2m, 2m+1) stacked on the PSUM partition dim (M=128):
6 accumulating bf16 matmuls (3 temporal taps x 2 source blocks) per
pair, with weight blocks pre-scattered host-side so every (v,u) tap
lands in exactly one matmul. Bias is fused into the PSUM->SBUF copy.
"""

import numpy as np
import ml_dtypes

import concourse.bacc as bacc
import concourse.mybir as mybir
from concourse.tile import TileContext
from concourse.bass_utils import run_bass_kernel_spmd

V, K, CIN, COUT, N, T, TK = 25, 3, 64, 64, 16, 512, 3
NCORES = 8
NPER = N // NCORES          # samples per core
TP = T + 2                  # block width incl. temporal zero pads
NB = (V + 1) // 2           # node-pair blocks
NSLOT = TK * V              # distinct [128,128] weight tiles

_BF16 = mybir.dt.bfloat16
_F32 = mybir.dt.float32

_cache = {}


def _build_program():
    nc = bacc.Bacc("TRN2", num_devices=NCORES)
    x_in = nc.dram_tensor("x", [NPER, CIN, V, T], _F32, kind="ExternalInput")
    wl_in = nc.dram_tensor("wl", [128, NSLOT * 128], _BF16, kind="ExternalInput")
    b_in = nc.dram_tensor("bias", [128, NB], _F32, kind="ExternalInput")
    y_out = nc.dram_tensor("y", [NPER, COUT, V, T], _F32, kind="ExternalOutput")

    with TileContext(nc) as tc:
        with (
            tc.tile_pool(name="w", bufs=1) as wp,
            tc.tile_pool(name="x", bufs=1) as xp,
            tc.tile_pool(name="ps", bufs=8, space="PSUM") as pp,
            tc.tile_pool(name="o", bufs=6) as op,
        ):
            wl_sb = wp.tile([128, NSLOT * 128], _BF16, tag="wl")
            nc.sync.dma_start(out=wl_sb[:, :], in_=wl_in[:, :])
            b_sb = wp.tile([128, NB], _F32, tag="bias")
            nc.sync.dma_start(out=b_sb[:, :], in_=b_in[:, :])

            xs = []
            for n in range(NPER):
                xf = xp.tile([128, NB * TP], _F32, tag=f"xf{n}")
                xfr = xf.rearrange("p (b w) -> p b w", w=TP)
                nc.vector.memset(xfr[:, :, 0:1], 0.0)      # left pads (t=-1)
                nc.vector.memset(xfr[:, :, 513:514], 0.0)  # right pads (t=T)
                nc.vector.memset(xf[0:64, 0:TP], 0.0)      # ghost node -1
                # even nodes 2j -> partitions 64-127, block j
                nc.sync.dma_start(out=xfr[64:128, :, 1:513], in_=x_in[n, :, ::2, :])
                # odd nodes 2j-1 -> partitions 0-63, blocks 1..12
                nc.sync.dma_start(out=xfr[0:64, 1:NB, 1:513], in_=x_in[n, :, 1::2, :])
                xsn = xp.tile([128, NB * TP], _BF16, tag=f"xs{n}")
                nc.vector.tensor_copy(out=xsn[:, :], in_=xf[:, :])
                xs.append(xsn)

            for n in range(NPER):
                for m in range(NB):
                    ps = pp.tile([128, 512], _F32)
                    taps = [
                        (kt, mmi)
                        for kt in range(TK)
                        for mmi in range(2)
                        if 2 * m + mmi < V
                    ]
                    for i, (kt, mmi) in enumerate(taps):
                        slot = kt * V + 2 * m + mmi
                        col = (m + mmi) * TP + kt
                        nc.tensor.matmul(
                            ps[:, :],
                            lhsT=wl_sb[:, slot * 128 : (slot + 1) * 128],
                            rhs=xs[n][:, col : col + 512],
                            start=(i == 0),
                            stop=(i == len(taps) - 1),
                        )
                    ot = op.tile([128, 512], _F32)
                    nc.vector.tensor_scalar_add(
                        out=ot[:, :], in0=ps[:, :], scalar1=b_sb[:, m : m + 1]
                    )
                    if 2 * m + 1 < V:
                        dst = y_out[n].rearrange("o v t -> v o t")[2 * m : 2 * m + 2]
                        nc.sync.dma_start(out=dst, in_=ot[:, :])
                    else:
                        nc.sync.dma_start(out=y_out[n, :, V - 1, :], in_=ot[0:64, :])

    nc.compile()
    return nc


def _prep_weights(W, b, idx, mask):
    W = np.asarray(W, np.float32)
    b = np.asarray(b, np.float32)
    idx = np.asarray(idx)
    mask = np.asarray(mask)
    Wm = np.where(mask[:, None, None, :, None], W, 0.0)  # [V,O,C,K,TK]
    W4 = np.zeros((V, V, COUT, CIN, TK), np.float32)
    for v in range(V):
        for k in range(K):
            if mask[v, k]:
                W4[v, idx[v, k]] = Wm[v, :, :, k, :]
    wl = np.zeros((128, NSLOT * 128), np.float32)
    for kt in range(TK):
        for s in range(V):
            m, mmi = s // 2, s % 2
            slot = kt * V + s
            blk = m + mmi
            for uh, u in ((0, 2 * blk - 1), (1, 2 * blk)):
                for vloc in range(2):
                    v = 2 * m + vloc
                    if 0 <= u < V and v < V:
                        # lhsT[64*uh + c, 64*vloc + o] = W4[v,u,o,c,kt]
                        wl[
                            64 * uh : 64 * uh + 64,
                            slot * 128 + 64 * vloc : slot * 128 + 64 * vloc + 64,
                        ] = W4[v, u, :, :, kt].T
    bias = np.zeros((128, NB), np.float32)
    for m in range(NB):
        for vloc in range(2):
            if 2 * m + vloc < V:
                bias[64 * vloc : 64 * vloc + 64, m] = b[2 * m + vloc]
    return wl.astype(ml_dtypes.bfloat16), bias


def kernel(x, W, b, idx, mask):
    x = np.ascontiguousarray(np.asarray(x, np.float32))
    if "nc" not in _cache:
        _cache["nc"] = _build_program()
    nc = _cache["nc"]
    wl, bias = _prep_weights(W, b, idx, mask)
    in_maps = [
        {"x": np.ascontiguousarray(x[c * NPER : (c + 1) * NPER]), "wl": wl, "bias": bias}
        for c in range(NCORES)
    ]
    res = run_bass_kernel_spmd(nc, in_maps, list(range(NCORES)))
    return np.concatenate([res.results[c]["y"] for c in range(NCORES)], axis=0)

